# revision 1
# baseline (speedup 1.0000x reference)
"""GQA attention (B=4, L=1024, D=4096, 32 Q heads / 8 KV heads, head_dim=128,
traditional RoPE, causal mask) on 8 TRN2 NeuronCores.

Sharding: tensor-parallel over heads. Core c owns Q heads {c, c+8, c+16, c+24}
(all map to KV head c under the reference's jnp.tile GQA expansion) — so each
core needs exactly one KV head. wq/wk/wv are column-sharded, wo row-sharded,
x replicated. Each core computes a partial output (its heads' contribution
through wo); the host sums the 8 partials.

On-chip layout: everything transposed. The projection computes q^T/k^T/v^T
([head_dim, tokens], head_dim on partitions) directly, which is exactly the
lhsT/rhs layout the scores matmul (s^T = k^T.T-contract) and the output
projection (lhsT = attn^T) want, so no activation-sized transposes are needed.
RoPE in transposed layout mixes partition pairs; that's done with one
128x128 pair-swap permutation matmul plus two elementwise muls against host
cos/sin tables. Softmax runs without max-subtraction (scores ~ N(0, 1.3^2)),
sums via a ones-vector matmul, normalization by an outer-product broadcast
of 1/sum. Causal structure: fully-masked 128x512 score blocks are skipped,
diagonal blocks are zeroed after exp with a binary mask (host-verified that
the mask is a pure 0/-1e9 mask).
"""

import numpy as np
import ml_dtypes
from contextlib import ExitStack

import concourse.bass as bass
import concourse.mybir as mybir
import concourse.tile as tile
from concourse import bacc
from concourse.bass_utils import run_bass_kernel_spmd

DIM = 4096
N_HEADS = 32
N_KV = 8
DH = 128
B, L = 4, 1024
NCORES = 8
HPC = N_HEADS // NCORES  # 4 q-heads per core
T = B * L  # 4096 tokens total
SCALE = DH ** -0.5
ROPE_BASE = 10000.0

BF = mybir.dt.bfloat16
F32 = mybir.dt.float32
NPBF = ml_dtypes.bfloat16

# number of 512-token q chunks per batch, 128-token k tiles per batch
QC = L // 512  # 2
KT = L // 128  # 8

TRACE = False
LAST_RESULT = [None]


def _classify_blocks(mask):
    """Per (kt, qc) block of mask^T: 'skip' (all masked), 'free' (no mask),
    or 'mixed'. Host-side; the kernel structure is specialized to this."""
    maskT = np.asarray(mask).T
    assert np.all((maskT == 0.0) | (maskT <= -1e8)), (
        "kernel assumes a binary additive mask (0 / -1e9)"
    )
    cls = {}
    for qc in range(QC):
        for kt in range(KT):
            blk = maskT[kt * 128:(kt + 1) * 128, qc * 512:(qc + 1) * 512]
            if np.all(blk <= -1e8):
                cls[(kt, qc)] = "skip"
            elif np.all(blk == 0.0):
                cls[(kt, qc)] = "free"
            else:
                cls[(kt, qc)] = "mixed"
    return cls


def _build(cls):
    nc = bacc.Bacc(
        "TRN2", target_bir_lowering=False, debug=False, num_devices=NCORES
    )

    # weights come in host-pre-tiled partition-major layout [128, ...] so
    # each loads as 128 large contiguous DMA descriptors
    NDT_ = DIM // 128
    xT = nc.dram_tensor("xT", [DIM, T], BF, kind="ExternalInput").ap()
    wq = nc.dram_tensor("wq", [128, NDT_ * HPC * DH], BF, kind="ExternalInput").ap()
    wk = nc.dram_tensor("wk", [128, NDT_ * DH], BF, kind="ExternalInput").ap()
    wv = nc.dram_tensor("wv", [128, NDT_ * DH], BF, kind="ExternalInput").ap()
    wo = nc.dram_tensor("wo", [128, HPC * DIM], BF, kind="ExternalInput").ap()
    mbinT = nc.dram_tensor("mbinT", [L, L], BF, kind="ExternalInput").ap()
    cos2 = nc.dram_tensor("cos2", [DH, L], BF, kind="ExternalInput").ap()
    sin2 = nc.dram_tensor("sin2", [DH, L], BF, kind="ExternalInput").ap()
    pswap = nc.dram_tensor("pswap", [DH, DH], BF, kind="ExternalInput").ap()
    ident = nc.dram_tensor("ident", [DH, DH], BF, kind="ExternalInput").ap()
    out = nc.dram_tensor("out", [T, DIM], F32, kind="ExternalOutput").ap()

    xT_r = xT.rearrange("(dt p) t -> dt p t", p=128)  # [32, 128, 4096]
    NDT = DIM // 128  # 32 contraction tiles

    with TileCtx(nc) as tc, ExitStack() as ctx:
        persist = ctx.enter_context(tc.tile_pool(name="persist", bufs=1))
        qt_pool = ctx.enter_context(tc.tile_pool(name="qt", bufs=HPC * B))
        kt_pool = ctx.enter_context(tc.tile_pool(name="kt", bufs=B))
        v_pool = ctx.enter_context(tc.tile_pool(name="v", bufs=B))

        cos_sb = persist.tile([DH, L], BF)
        sin_sb = persist.tile([DH, L], BF)
        psw_sb = persist.tile([DH, DH], BF)
        idn_sb = persist.tile([DH, DH], BF)
        ones_sb = persist.tile([128, 128], BF)
        nc.vector.memset(ones_sb, 1.0)
        nc.sync.dma_start(out=cos_sb, in_=cos2)
        nc.sync.dma_start(out=sin_sb, in_=sin2)
        nc.sync.dma_start(out=psw_sb, in_=pswap)
        nc.sync.dma_start(out=idn_sb, in_=ident)

        # wo + mask tiles live in outer pools (created before stage A's pools)
        # so their SBUF addresses don't overlap stage-A tiles; their DMAs are
        # emitted at the end of stage A so they don't delay the A-critical
        # weight/x loads at kernel start.
        wo_p = ctx.enter_context(tc.tile_pool(name="wo_p", bufs=1))
        mp = ctx.enter_context(tc.tile_pool(name="mp", bufs=8))
        wo_sb = wo_p.tile([128, HPC, DIM], BF)
        msk_sb = {}
        for (kt, qc), c in cls.items():
            if c == "mixed":
                msk_sb[(kt, qc)] = mp.tile([128, 512], BF, name="mtile")

        qt_t = [[None] * B for _ in range(HPC)]  # [128 dh, 1024 t] per (h, b)
        kt_t = [None] * B                        # [128 dh, 1024 t]
        v_t = [None] * B                         # [128 t, 8, 128 dh]

        # ---------------- Stage A: QKV projection + RoPE ----------------
        with tc.tile_pool(name="wA", bufs=1) as wA, \
             tc.tile_pool(name="xp", bufs=8) as xp, \
             tc.tile_pool(name="evac", bufs=8) as evac, \
             tc.tile_pool(name="rtmp", bufs=8) as rtmp, \
             tc.tile_pool(name="psA", bufs=6, space="PSUM") as psA, \
             tc.tile_pool(name="psS", bufs=1, space="PSUM") as psS:

            wq_sb = wA.tile([128, NDT, HPC * DH], BF)
            wk_sb = wA.tile([128, NDT, DH], BF)
            wv_sb = wA.tile([128, NDT, DH], BF)
            nc.sync.dma_start(out=wk_sb, in_=wk.rearrange("p (dt m) -> p dt m", dt=NDT))
            nc.sync.dma_start(out=wv_sb, in_=wv.rearrange("p (dt m) -> p dt m", dt=NDT))
            nc.sync.dma_start(out=wq_sb, in_=wq.rearrange("p (dt m) -> p dt m", dt=NDT))

            for tci in range(T // 512):  # 8 chunks of 512 tokens
                b, half = tci // 2, tci % 2
                lsl = slice(half * 512, (half + 1) * 512)  # pos within batch
                if half == 0:
                    for h in range(HPC):
                        qt_t[h][b] = qt_pool.tile([DH, L], BF, name="qtile")
                    kt_t[b] = kt_pool.tile([DH, L], BF, name="ktile")
                    v_t[b] = v_pool.tile([128, KT, DH], BF, name="vtile")

                ps_q = [psA.tile([128, 512], F32, name="psacc") for _ in range(HPC)]
                ps_k = psA.tile([128, 512], F32, name="psacc")
                ps_v = psA.tile([128, 512], F32, name="psacc")
                for d in range(NDT):
                    xt = xp.tile([128, 512], BF)
                    nc.sync.dma_start(
                        out=xt, in_=xT_r[d, :, tci * 512:(tci + 1) * 512]
                    )
                    st, sp = d == 0, d == NDT - 1
                    for h in range(HPC):
                        nc.tensor.matmul(
                            ps_q[h], wq_sb[:, d, h * DH:(h + 1) * DH], xt,
                            start=st, stop=sp,
                        )
                    nc.tensor.matmul(ps_k, wk_sb[:, d], xt, start=st, stop=sp)
                    nc.tensor.matmul(ps_v, wv_sb[:, d], xt, start=st, stop=sp)

                # RoPE on q heads and k: r = raw*cos + (P raw)*sin
                for h in range(HPC + 1):
                    ps = ps_k if h == HPC else ps_q[h]
                    dst = kt_t[b] if h == HPC else qt_t[h][b]
                    raw = evac.tile([128, 512], BF, name="raw")
                    nc.scalar.copy(raw, ps)
                    ps_sw = psS.tile([128, 512], F32, name="pssw")
                    nc.tensor.matmul(ps_sw, psw_sb, raw, start=True, stop=True)
                    t1 = rtmp.tile([128, 512], BF, name="t1")
                    t2 = rtmp.tile([128, 512], BF, name="t2")
                    nc.vector.tensor_mul(t1, raw, cos_sb[:, lsl])
                    nc.vector.tensor_mul(t2, ps_sw, sin_sb[:, lsl])
                    nc.vector.tensor_add(dst[:, lsl], t1, t2)

                # v: transpose [dh, t] -> [t, dh] natural, 128 cols at a time
                vraw = evac.tile([128, 512], BF, name="raw")
                nc.scalar.copy(vraw, ps_v)
                for s in range(4):
                    ps_t = psS.tile([128, 128], BF, name="pstr")
                    nc.tensor.transpose(ps_t, vraw[:, s * 128:(s + 1) * 128], idn_sb)
                    nc.vector.tensor_copy(v_t[b][:, half * 4 + s], ps_t)

                if tci == 0:
                    # B/C-stage constants: emitted here (not at kernel start)
                    # so they queue behind the A-critical first loads.
                    nc.sync.dma_start(
                        out=wo_sb, in_=wo.rearrange("p (h n) -> p h n", h=HPC)
                    )
                    for (kt, qc), m in msk_sb.items():
                        nc.sync.dma_start(
                            out=m,
                            in_=mbinT[
                                kt * 128:(kt + 1) * 128, qc * 512:(qc + 1) * 512
                            ],
                        )

        # ---------------- Stage B + C: attention + output proj ----------------
        with tc.tile_pool(name="ep", bufs=8) as ep, \
             tc.tile_pool(name="attn", bufs=2 * HPC) as attn_p, \
             tc.tile_pool(name="rcp", bufs=2) as rcp, \
             tc.tile_pool(name="oev", bufs=4) as oev, \
             tc.tile_pool(name="psMM", bufs=4, space="PSUM") as psMM, \
             tc.tile_pool(name="psPV", bufs=2, space="PSUM") as psPV, \
             tc.tile_pool(name="psSum", bufs=2, space="PSUM") as psSum:

            attn_t = [[None] * B for _ in range(HPC)]
            for b in range(B):
                for h in range(HPC):
                    at = attn_p.tile([DH, L], BF, name="atile")
                    attn_t[h][b] = at
                    for qc in range(QC):
                        kts = [k for k in range(KT) if cls[(k, qc)] != "skip"]
                        ps_pv = psPV.tile([128, 512], F32, name="pspv")
                        # ones[128,128] lhsT -> every partition gets the k-sum
                        # row, so normalization is a plain elementwise mul
                        # (no 1-partition reciprocal, no broadcast matmul).
                        ps_sum = psSum.tile([128, 512], F32, name="pssum")
                        for gi in range(0, len(kts), 4):
                            grp = kts[gi:gi + 4]
                            e_ts, ps_ss = {}, {}
                            for kt in grp:
                                ps_s = psMM.tile([128, 512], F32, name="mmps")
                                nc.tensor.matmul(
                                    ps_s,
                                    kt_t[b][:, kt * 128:(kt + 1) * 128],
                                    qt_t[h][b][:, qc * 512:(qc + 1) * 512],
                                    start=True, stop=True,
                                )
                                ps_ss[kt] = ps_s
                            for kt in grp:
                                e_t = ep.tile([128, 512], BF, name="etile")
                                nc.scalar.activation(
                                    e_t, ps_ss[kt],
                                    mybir.ActivationFunctionType.Exp,
                                    scale=SCALE,
                                )
                                if cls[(kt, qc)] == "mixed":
                                    nc.vector.tensor_mul(e_t, e_t, msk_sb[(kt, qc)])
                                e_ts[kt] = e_t
                            for kt in grp:
                                st = kt == kts[0]
                                sp = kt == kts[-1]
                                nc.tensor.matmul(
                                    ps_pv, v_t[b][:, kt], e_ts[kt],
                                    start=st, stop=sp,
                                )
                                nc.tensor.matmul(
                                    ps_sum, ones_sb, e_ts[kt],
                                    start=st, stop=sp,
                                )
                        recip = rcp.tile([128, 512], F32, name="recip")
                        nc.vector.reciprocal_approx_fast(recip, ps_sum)
                        nc.vector.tensor_mul(
                            at[:, qc * 512:(qc + 1) * 512], ps_pv, recip
                        )

                # Stage C for batch b: out[t, :] += sum_h attn_h @ wo_h
                for tt in range(KT):  # 8 token tiles of 128
                    for nck in range(DIM // 512):  # 8 n chunks
                        ps_o = psMM.tile([128, 512], F32, name="mmps")
                        for h in range(HPC):
                            nc.tensor.matmul(
                                ps_o,
                                attn_t[h][b][:, tt * 128:(tt + 1) * 128],
                                wo_sb[:, h, nck * 512:(nck + 1) * 512],
                                start=(h == 0), stop=(h == HPC - 1),
                            )
                        o_sb = oev.tile([128, 512], F32, name="osb")
                        if (tt + nck) % 2 == 0:
                            nc.vector.tensor_copy(o_sb, ps_o)
                        else:
                            nc.scalar.copy(o_sb, ps_o)
                        nc.sync.dma_start(
                            out=out[
                                b * L + tt * 128: b * L + (tt + 1) * 128,
                                nck * 512:(nck + 1) * 512,
                            ],
                            in_=o_sb,
                        )
    nc.finalize()
    return nc


def TileCtx(nc):
    return tile.TileContext(nc)


def _host_tables():
    inv = ROPE_BASE ** (-np.arange(0, DH, 2, dtype=np.float64) / DH)  # [64]
    pos = np.arange(L, dtype=np.float64)
    ang = inv[:, None] * pos[None, :]  # [64, L]
    cos2 = np.repeat(np.cos(ang), 2, axis=0)  # [128, L]
    sin = np.sin(ang)
    sin2 = np.empty((DH, L), dtype=np.float64)
    sin2[0::2] = -sin
    sin2[1::2] = sin
    psw = np.zeros((DH, DH), dtype=np.float32)
    idx = np.arange(0, DH, 2)
    psw[idx, idx + 1] = 1.0
    psw[idx + 1, idx] = 1.0
    return (
        cos2.astype(NPBF),
        sin2.astype(NPBF),
        psw.astype(NPBF),
        np.eye(DH, dtype=np.float32).astype(NPBF),
    )


def kernel(x, mask, wq, wk, wv, wo):
    x = np.asarray(x, dtype=np.float32)
    mask = np.asarray(mask, dtype=np.float32)
    wq = np.asarray(wq, dtype=np.float32)
    wk = np.asarray(wk, dtype=np.float32)
    wv = np.asarray(wv, dtype=np.float32)
    wo = np.asarray(wo, dtype=np.float32)

    cls = _classify_blocks(mask)
    nc = _build(cls)

    xT = np.ascontiguousarray(x.reshape(T, DIM).T).astype(NPBF)
    mbinT = np.ascontiguousarray((mask == 0.0).T.astype(NPBF))
    cos2, sin2, psw, idn = _host_tables()

    def _ptile(w):
        # [DIM_or_512, M] -> partition-major [128, (outer M)] host pre-tiling
        k, m = w.shape
        return np.ascontiguousarray(
            w.reshape(k // 128, 128, m).transpose(1, 0, 2).reshape(128, -1)
        ).astype(NPBF)

    in_maps = []
    for c in range(NCORES):
        cols = np.concatenate(
            [np.arange(h * DH, (h + 1) * DH) for h in range(c, N_HEADS, N_KV)]
        )
        in_maps.append({
            "xT": xT,
            "wq": _ptile(wq[:, cols]),
            "wk": _ptile(wk[:, c * DH:(c + 1) * DH]),
            "wv": _ptile(wv[:, c * DH:(c + 1) * DH]),
            "wo": _ptile(wo[cols, :]),
            "mbinT": mbinT,
            "cos2": cos2,
            "sin2": sin2,
            "pswap": psw,
            "ident": idn,
        })

    res = run_bass_kernel_spmd(
        nc, in_maps, core_ids=list(range(NCORES)), trace=TRACE
    )
    LAST_RESULT[0] = res
    outs = res.results
    total = np.zeros((T, DIM), dtype=np.float32)
    for c in range(NCORES):
        total += np.asarray(outs[c]["out"], dtype=np.float32)
    return total.reshape(B, L, DIM)



# revision 4
# speedup vs baseline: 1.0275x; 1.0275x over previous
"""GQA attention (B=4, L=1024, D=4096, 32 Q heads / 8 KV heads, head_dim=128,
traditional RoPE, causal mask) on 8 TRN2 NeuronCores.

Sharding: tensor-parallel over heads. Core c owns Q heads {c, c+8, c+16, c+24}
(all map to KV head c) - each core needs exactly one KV head. wq/wk/wv
column-sharded, wo row-sharded, x replicated. Each core computes a partial
output through wo; the host sums the 8 partials (and transposes: the kernel
writes out^T [DIM, T] in bf16).

v2 changes vs baseline:
- RoPE with zero tensor-engine work: wq/wk columns are host-permuted so the
  rotation pairs (2i, 2i+1) land at partitions (i, 64+i). The pair-swap then
  becomes two half-partition DVE adds against contiguous halves - no 128x128
  permutation matmuls, no extra PSUM bank, no head-of-line stalls.
- Startup: weights stream in per-d-slice DMAs interleaved with the first x
  chunk, so the first matmul issues ~1us in instead of ~31us.
- Attention uses 256-token q chunks (finer causal skip: 10 vs 12 block-equivs
  per head) and 4 q-head scores per PSUM group tile, exp'd in one wide
  ACT instruction spanning banks.
- Output projection keeps wo stationary and streams attn, accumulating both
  512-token halves per LDWEIGHTS; output is written transposed in bf16,
  halving the write traffic.
"""

import numpy as np
import ml_dtypes
from contextlib import ExitStack

import concourse.bass as bass
import concourse.mybir as mybir
import concourse.tile as tile
from concourse import bacc
from concourse.bass_utils import run_bass_kernel_spmd

DIM = 4096
N_HEADS = 32
N_KV = 8
DH = 128
B, L = 4, 1024
NCORES = 8
HPC = N_HEADS // NCORES  # 4 q-heads per core
T = B * L  # 4096 tokens total
SCALE = DH ** -0.5
ROPE_BASE = 10000.0
NDT = DIM // 128  # 32 contraction tiles

BF = mybir.dt.bfloat16
F32 = mybir.dt.float32
NPBF = ml_dtypes.bfloat16

QC4 = L // 256  # 4 q-chunks of 256 per batch
KT = L // 128   # 8 k tiles of 128 per batch

TRACE = False
LAST_RESULT = [None]


def _check_mask(mask):
    """Verify the mask is the binary causal mask this kernel is specialized
    to, and return the 8 diagonal 128x256 keep-patterns [128, 8, 256]."""
    m = np.asarray(mask)
    assert m.shape == (L, L)
    assert np.all((m == 0.0) | (m <= -1e8)), "kernel assumes binary additive mask"
    keep = (m == 0.0)  # [q, k]
    dmask = np.zeros((128, KT, 256), dtype=np.float32)
    for qc in range(QC4):
        qs = slice(qc * 256, (qc + 1) * 256)
        for kt in range(KT):
            ks = slice(kt * 128, (kt + 1) * 128)
            blk = keep[qs, ks]  # [256 q, 128 k]
            if kt > 2 * qc + 1:
                assert not blk.any(), f"block ({kt},{qc}) expected fully masked"
            elif kt < 2 * qc:
                assert blk.all(), f"block ({kt},{qc}) expected free"
            else:
                dmask[:, kt, :] = blk.T  # [128 k, 256 q]
    return dmask


def _build():
    nc = bacc.Bacc(
        "TRN2", target_bir_lowering=False, debug=False, num_devices=NCORES
    )

    xT = nc.dram_tensor("xT", [DIM, T], BF, kind="ExternalInput").ap()
    # all QKV weights in one tensor, d-major: [:, d] = [wv_d | wk_d | wq_d]
    # (128+128+512 cols) so a 4-d chunk loads as ONE large dma_start
    wall = nc.dram_tensor("wall", [128, NDT * 768], BF, kind="ExternalInput").ap()
    wo = nc.dram_tensor("wo", [128, HPC * DIM], BF, kind="ExternalInput").ap()
    cosq = nc.dram_tensor("cosq", [128, 2 * HPC * 512], BF, kind="ExternalInput").ap()
    sinq = nc.dram_tensor("sinq", [128, 2 * HPC * 512], BF, kind="ExternalInput").ap()
    cosk = nc.dram_tensor("cosk", [128, L], BF, kind="ExternalInput").ap()
    sink = nc.dram_tensor("sink", [128, L], BF, kind="ExternalInput").ap()
    ident = nc.dram_tensor("ident", [DH, DH], BF, kind="ExternalInput").ap()
    dmask = nc.dram_tensor("dmask", [128, KT * 256], BF, kind="ExternalInput").ap()
    out = nc.dram_tensor("out", [DIM, T], BF, kind="ExternalOutput").ap()

    xT_r = xT.rearrange("(dt p) t -> dt p t", p=128)  # [32, 128, 4096]
    wall_r = wall.rearrange("p (dt m) -> p dt m", dt=NDT)  # [128, 32, 768]

    with tile.TileContext(nc) as tc, ExitStack() as ctx:
        persist = ctx.enter_context(tc.tile_pool(name="persist", bufs=1))
        qt_pool = ctx.enter_context(tc.tile_pool(name="qt", bufs=B))
        kt_pool = ctx.enter_context(tc.tile_pool(name="kt", bufs=B))
        v_pool = ctx.enter_context(tc.tile_pool(name="v", bufs=B))
        wo_p = ctx.enter_context(tc.tile_pool(name="wo_p", bufs=1))

        ones_sb = persist.tile([128, 128], BF)
        nc.vector.memset(ones_sb, 1.0)
        cosq_sb = persist.tile([128, 2, HPC, 512], BF)
        sinq_sb = persist.tile([128, 2, HPC, 512], BF)
        cosk_sb = persist.tile([128, L], BF)
        sink_sb = persist.tile([128, L], BF)
        idn_sb = persist.tile([DH, DH], BF)
        dmsk_sb = persist.tile([128, KT, 256], BF)
        wo_sb = wo_p.tile([128, HPC, DIM], BF)

        qt_all = [None] * B  # [128 dh, HPC, 1024] rope'd q, halves layout
        kt_all = [None] * B  # [128 dh, 1024]
        v_t = [None] * B     # [128 t, KT, 128 dh]

        # ---------------- Stage A: QKV projection + RoPE ----------------
        with tc.tile_pool(name="wA", bufs=1) as wA, \
             tc.tile_pool(name="xp", bufs=8) as xp, \
             tc.tile_pool(name="evq", bufs=2) as evq, \
             tc.tile_pool(name="evs", bufs=2) as evs, \
             tc.tile_pool(name="rtmp", bufs=1) as rtmp, \
             tc.tile_pool(name="psA", bufs=1, space="PSUM") as psA, \
             tc.tile_pool(name="psT", bufs=2, space="PSUM") as psT:

            wall_sb = wA.tile([128, NDT, 768], BF)

            def _wv(d):
                return wall_sb[:, d, 0:DH]

            def _wk(d):
                return wall_sb[:, d, DH:2 * DH]

            def _wq(d, h):
                return wall_sb[:, d, 2 * DH + h * DH:2 * DH + (h + 1) * DH]

            # weight chunks of 4 d-slices, one dma_start each; two issued
            # before the loop so the stream stays ahead of compute
            def _wchunk(k):
                dsl = slice(4 * k, 4 * k + 4)
                nc.sync.dma_start(out=wall_sb[:, dsl], in_=wall_r[:, dsl])

            _wchunk(0)
            _wchunk(1)

            for tci in range(T // 512):  # 8 chunks of 512 tokens
                b, half = tci // 2, tci % 2
                lsl = slice(half * 512, (half + 1) * 512)
                if half == 0:
                    qt_all[b] = qt_pool.tile([128, HPC, L], BF, name="qtile")
                    kt_all[b] = kt_pool.tile([128, L], BF, name="ktile")
                    v_t[b] = v_pool.tile([128, KT, DH], BF, name="vtile")

                # bank order: v(0), k(1), q(2..5)
                ps_v = psA.tile([128, 512], F32, name="psv")
                ps_k = psA.tile([128, 512], F32, name="psk")
                ps_q = psA.tile([128, HPC, 512], F32, name="psq")

                prefetched = {}
                if tci == 0:
                    # x prefetch for the first d's goes out before the PE
                    # warmup so the sync ring is full from t=0
                    for d in range(4):
                        xt = xp.tile([128, 512], BF)
                        nc.sync.dma_start(out=xt, in_=xT_r[d, :, 0:512])
                        prefetched[d] = xt
                    # HAM warmup: keep the PE busy while DMAs land so the
                    # clock gate opens before the real stream begins
                    for wu in range(24):
                        nc.tensor.matmul(
                            ps_q[:, 0, 0:128], ones_sb, ones_sb,
                            start=True, stop=True,
                        )

                for d in range(NDT):
                    if tci == 0:
                        if d % 4 == 1 and d // 4 + 2 < 8:
                            _wchunk(d // 4 + 2)
                        if d == 4:
                            # bulk constants go on the scalar DMA ring so they
                            # don't block the x/weight stream on the sync ring
                            nc.scalar.dma_start(
                                out=cosq_sb,
                                in_=cosq.rearrange(
                                    "p (a h t) -> p a h t", a=2, h=HPC
                                ),
                            )
                            nc.scalar.dma_start(
                                out=sinq_sb,
                                in_=sinq.rearrange(
                                    "p (a h t) -> p a h t", a=2, h=HPC
                                ),
                            )
                            nc.scalar.dma_start(out=cosk_sb, in_=cosk)
                            nc.scalar.dma_start(out=sink_sb, in_=sink)
                            nc.scalar.dma_start(out=idn_sb, in_=ident)
                            nc.scalar.dma_start(
                                out=dmsk_sb,
                                in_=dmask.rearrange("p (k t) -> p k t", k=KT),
                            )
                    if tci == 1 and d == 0:
                        nc.scalar.dma_start(
                            out=wo_sb, in_=wo.rearrange("p (h n) -> p h n", h=HPC)
                        )
                    if d in prefetched:
                        xt = prefetched.pop(d)
                    else:
                        xt = xp.tile([128, 512], BF)
                        nc.sync.dma_start(
                            out=xt, in_=xT_r[d, :, tci * 512:(tci + 1) * 512]
                        )
                    st, sp = d == 0, d == NDT - 1
                    nc.tensor.matmul(ps_v, _wv(d), xt, start=st, stop=sp)
                    nc.tensor.matmul(ps_k, _wk(d), xt, start=st, stop=sp)
                    for h in range(HPC):
                        nc.tensor.matmul(
                            ps_q[:, h], _wq(d, h), xt, start=st, stop=sp,
                        )

                # --- tail: evacuate + v-transpose + RoPE (no PE rope work) ---
                # evac split across ACT (vraw, q01) and DVE (kraw, q23) so
                # PSUM banks free in a staggered pattern for the next tci
                vraw = evs.tile([128, 512], BF, name="vraw")
                nc.scalar.copy(vraw, ps_v)
                for s in range(4):
                    ps_t = psT.tile([128, 128], BF, name="pstr")
                    nc.tensor.transpose(ps_t, vraw[:, s * 128:(s + 1) * 128], idn_sb)
                    nc.vector.tensor_copy(v_t[b][:, half * 4 + s], ps_t)

                kraw = evs.tile([128, 512], BF, name="kraw")
                nc.vector.tensor_copy(kraw, ps_k)
                qraw = evq.tile([128, HPC, 512], BF, name="qraw")
                nc.scalar.copy(qraw[:, 0:2], ps_q[:, 0:2])
                nc.vector.tensor_copy(qraw[:, 2:4], ps_q[:, 2:4])

                # RoPE: dst = raw*cos + swap64(raw*sinSw)  (halves layout).
                # TT inputs must share a base partition (NCC_IBIR297), so the
                # 64-half swap goes through single-input copies.
                u_k = rtmp.tile([128, 512], BF, name="uk")
                t_k = rtmp.tile([128, 512], BF, name="tk")
                usw_k = rtmp.tile([128, 512], BF, name="uswk")
                nc.vector.tensor_mul(u_k, kraw, sink_sb[:, lsl])
                nc.vector.tensor_mul(t_k, kraw, cosk_sb[:, lsl])
                nc.vector.tensor_copy(usw_k[0:64], u_k[64:128])
                nc.vector.tensor_copy(usw_k[64:128], u_k[0:64])
                nc.vector.tensor_add(kt_all[b][:, lsl], t_k, usw_k)

                u_q = rtmp.tile([128, HPC, 512], BF, name="uq")
                t_q = rtmp.tile([128, HPC, 512], BF, name="tq")
                usw_q = rtmp.tile([128, HPC, 512], BF, name="uswq")
                nc.vector.tensor_mul(u_q, qraw, sinq_sb[:, half])
                nc.vector.tensor_mul(t_q, qraw, cosq_sb[:, half])
                nc.vector.tensor_copy(usw_q[0:64], u_q[64:128])
                nc.vector.tensor_copy(usw_q[64:128], u_q[0:64])
                nc.vector.tensor_add(qt_all[b][:, :, lsl], t_q, usw_q)

        # ---------------- Stages B + C, per batch ----------------
        # one pool set for all batches: no pool open/close barriers between
        # B(b) -> C(b) -> B(b+1); C's output tiles share the psS tag (same
        # 2-bank slot size), so the PE flows straight across stages.
        attn_pool = ctx.enter_context(tc.tile_pool(name="attn", bufs=2))
        ep = ctx.enter_context(tc.tile_pool(name="ep", bufs=4))
        rcp = ctx.enter_context(tc.tile_pool(name="rcp", bufs=2))
        oev = ctx.enter_context(tc.tile_pool(name="oev", bufs=4))
        psS = ctx.enter_context(tc.tile_pool(name="psS", bufs=2, space="PSUM"))
        psPV = ctx.enter_context(tc.tile_pool(name="psPV", bufs=2, space="PSUM"))
        psSum = ctx.enter_context(tc.tile_pool(name="psSum", bufs=2, space="PSUM"))
        attn_all = [None] * B

        for b in range(B):
            attn_all[b] = attn_pool.tile([128, HPC, L], BF, name="atile")

            # ---- B(b): attention, software-pipelined one group ahead so
            # the PE always has the next group's score matmuls queued while
            # ACT runs exp on the previous group ----
            for h in range(HPC):
                # sequence of (qc, group, first_of_qc, last_of_qc)
                seq = []
                for qc in range(QC4):
                    kts = list(range(2 * qc + 2))
                    groups = [g for g in (kts[0:4], kts[4:]) if g]
                    for gi, g in enumerate(groups):
                        seq.append((qc, g, gi == 0, gi == len(groups) - 1))

                pvs = {}  # qc -> (pv tile, sum tile); separate banks: a
                # start=True matmul clears its WHOLE bank, so pv and sum
                # must not share one
                pending = None  # (qc, group, e_g, first, last)

                def flush_pv(item):
                    qc, g, e_g, e2, first, last = item
                    if first:
                        pvs[qc] = (
                            psPV.tile([128, 256], F32, name="pspv",
                                      padded_shape=[128, 512]),
                            psSum.tile([128, 256], F32, name="pssum",
                                       padded_shape=[128, 512]),
                        )
                    pv, psum = pvs[qc]
                    for i, kt in enumerate(g):
                        st = first and i == 0
                        sp = last and i == len(g) - 1
                        nc.tensor.matmul(
                            pv, v_t[b][:, kt], e_g[:, i],
                            start=st, stop=sp,
                        )
                    np2 = len(g) // 2
                    for j in range(np2):
                        st = first and j == 0
                        sp = last and j == np2 - 1
                        nc.tensor.matmul(
                            psum, ones_sb, e2[:, j],
                            start=st, stop=sp,
                        )
                    if last:
                        q_sl = slice(qc * 256, (qc + 1) * 256)
                        recip = rcp.tile([128, 256], F32, name="recip")
                        nc.vector.reciprocal_approx_fast(recip, psum)
                        nc.vector.tensor_mul(
                            attn_all[b][:, h, q_sl], pv, recip
                        )
                        del pvs[qc]

                for qc, g, first, last in seq:
                    q_sl = slice(qc * 256, (qc + 1) * 256)
                    s = len(g)
                    ps_s = psS.tile(
                        [128, s, 256], F32, name="pss",
                        padded_shape=[128, 4, 256],
                    )
                    for i, kt in enumerate(g):
                        nc.tensor.matmul(
                            ps_s[:, i],
                            kt_all[b][:, kt * 128:(kt + 1) * 128],
                            qt_all[b][:, h, q_sl],
                            start=True, stop=True,
                        )
                    e_g = ep.tile([128, s, 256], BF, name="etile")
                    nc.scalar.activation(
                        e_g, ps_s,
                        mybir.ActivationFunctionType.Exp,
                        scale=SCALE,
                    )
                    if last:
                        nc.vector.tensor_mul(
                            e_g[:, s - 2:s], e_g[:, s - 2:s],
                            dmsk_sb[:, 2 * qc:2 * qc + 2],
                        )
                    # pair-sum on DVE so the softmax denominator needs one
                    # ones-matmul per kt PAIR instead of per kt
                    e2 = ep.tile([128, s // 2, 256], BF, name="e2tile")
                    nc.vector.tensor_add(
                        e2, e_g[:, 0:s:2], e_g[:, 1:s:2]
                    )
                    if pending is not None:
                        flush_pv(pending)
                    pending = (qc, g, e_g, e2, first, last)
                flush_pv(pending)

            # ---- C(b): output projection, wo stationary, out^T ----
            if True:
                for nb in range(DIM // 128):  # 32 blocks of 128 output cols
                    ps_c = psS.tile([128, 2, 512], F32, name="pss")
                    for h in range(HPC):
                        for t2 in range(2):
                            nc.tensor.matmul(
                                ps_c[:, t2],
                                wo_sb[:, h, nb * 128:(nb + 1) * 128],
                                attn_all[b][:, h, t2 * 512:(t2 + 1) * 512],
                                start=(h == 0), stop=(h == HPC - 1),
                            )
                    o_sb = oev.tile([128, 1024], BF, name="osb")
                    if nb % 2 == 0:
                        nc.vector.tensor_copy(o_sb, ps_c)
                    else:
                        nc.scalar.copy(o_sb, ps_c)
                    nc.sync.dma_start(
                        out=out[nb * 128:(nb + 1) * 128, b * L:(b + 1) * L],
                        in_=o_sb,
                    )

    nc.finalize()
    return nc


def _host_tables():
    """cos/sin tables in the halves layout: row i (i<64) = even dim 2i,
    row 64+i = odd dim 2i+1. u = raw*sinSw; dst_lo = t1_lo + u_hi needs
    sinSw = [+sin; -sin]; cosH = [cos; cos]."""
    inv = ROPE_BASE ** (-np.arange(0, DH, 2, dtype=np.float64) / DH)  # [64]
    pos = np.arange(L, dtype=np.float64)
    ang = inv[:, None] * pos[None, :]  # [64, L]
    cosA, sinA = np.cos(ang), np.sin(ang)
    cosH = np.concatenate([cosA, cosA], axis=0)  # [128, L]
    sinSw = np.concatenate([sinA, -sinA], axis=0)  # [128, L]
    # q tables: [128, 2 halves, HPC, 512] with the same positional slice
    # repeated across heads
    cosq = np.empty((128, 2, HPC, 512), dtype=np.float64)
    sinq = np.empty((128, 2, HPC, 512), dtype=np.float64)
    for half in range(2):
        sl = slice(half * 512, (half + 1) * 512)
        cosq[:, half] = cosH[:, sl][:, None, :]
        sinq[:, half] = sinSw[:, sl][:, None, :]
    return (
        cosq.reshape(128, -1).astype(NPBF),
        sinq.reshape(128, -1).astype(NPBF),
        np.ascontiguousarray(cosH).astype(NPBF),
        np.ascontiguousarray(sinSw).astype(NPBF),
    )


def _ptile(w):
    # [K, M] -> partition-major [128, (K/128, M)] host pre-tiling
    k, m = w.shape
    return np.ascontiguousarray(
        w.reshape(k // 128, 128, m).transpose(1, 0, 2).reshape(128, -1)
    ).astype(NPBF)


def kernel(x, mask, wq, wk, wv, wo):
    x = np.asarray(x, dtype=np.float32)
    mask = np.asarray(mask, dtype=np.float32)
    wq = np.asarray(wq, dtype=np.float32)
    wk = np.asarray(wk, dtype=np.float32)
    wv = np.asarray(wv, dtype=np.float32)
    wo = np.asarray(wo, dtype=np.float32)

    dmask = _check_mask(mask)
    nc = _build()

    xT = np.ascontiguousarray(x.reshape(T, DIM).T).astype(NPBF)
    cosq, sinq, cosk, sink = _host_tables()
    idn = np.eye(DH, dtype=np.float32).astype(NPBF)
    dmask_in = np.ascontiguousarray(dmask.reshape(128, -1)).astype(NPBF)

    # halves permutation of the head_dim axis: even dims then odd dims
    perm = np.concatenate([np.arange(0, DH, 2), np.arange(1, DH, 2)])

    in_maps = []
    for c in range(NCORES):
        cols = np.concatenate(
            [np.arange(h * DH, (h + 1) * DH) for h in range(c, N_HEADS, N_KV)]
        )
        wq_c = wq[:, cols].reshape(DIM, HPC, DH)[:, :, perm].reshape(DIM, -1)
        wk_c = wk[:, c * DH:(c + 1) * DH][:, perm]
        wv_c = wv[:, c * DH:(c + 1) * DH]
        # d-major wall [128, NDT, 768]: [:, d] = [wv_d | wk_d | wq_d]
        pv_, pk_, pq_ = (
            _ptile(wv_c).reshape(128, NDT, DH),
            _ptile(wk_c).reshape(128, NDT, DH),
            _ptile(wq_c).reshape(128, NDT, HPC * DH),
        )
        wall = np.concatenate([pv_, pk_, pq_], axis=2).reshape(128, -1)
        in_maps.append({
            "xT": xT,
            "wall": np.ascontiguousarray(wall),
            "wo": _ptile(wo[cols, :]),
            "cosq": cosq,
            "sinq": sinq,
            "cosk": cosk,
            "sink": sink,
            "ident": idn,
            "dmask": dmask_in,
        })

    res = run_bass_kernel_spmd(
        nc, in_maps, core_ids=list(range(NCORES)), trace=TRACE
    )
    LAST_RESULT[0] = res
    outs = res.results
    total = np.zeros((DIM, T), dtype=np.float32)
    for c in range(NCORES):
        total += np.asarray(outs[c]["out"], dtype=np.float32)
    return np.ascontiguousarray(total.T).reshape(B, L, DIM)
